# revision 1
# baseline (speedup 1.0000x reference)
"""DigitCapsules dynamic-routing kernel for 8 TRN2 NeuronCores.

Strategy (hardcoded for B=128, R=2048, O=16, D=16, C=16, 3 routing iters):
  - Shard R across the 8 cores (256 routes/core). W shard = 4.2 MB -> SBUF
    resident in f16; x replicated.
  - u_hat (= x @ W) generated once on the TensorEngine (K=16 matmuls packed
    4x via row tile_position) and kept SBUF-resident as f16
    [b=128 partitions, (o, c, r)] with r innermost (dense for DVE 2x mode).
  - Routing iterations on the VectorEngine: softmax over the global R needs
    only a per-(b,o) sum of exp(b_ij); b_ij stays in [-0.14, 0.35] so no max
    subtraction is needed.  Cross-core denominator = one 8 KB AllReduce per
    iteration (iters 1,2).
  - Contractions over O (weighted by c_ij) and over C (agreement with v) are
    strided pairwise tree-adds in f16 (DVE 2x mode).
  - Output v is returned per-core as [b, c, r_loc] f32 and assembled on host.
"""

import os
import sys

import numpy as np

for _p in ("/opt/trn_rl_repo", "/root/.axon_site/_ro/trn_rl_repo"):
    if os.path.isdir(_p) and _p not in sys.path:
        sys.path.insert(0, _p)

import concourse.bass as bass  # noqa: E402
from concourse import bacc  # noqa: E402
import concourse.tile as tile  # noqa: E402
from concourse import mybir  # noqa: E402
from concourse import bass_utils  # noqa: E402

B, R, O, D, C = 128, 2048, 16, 16, 16
NCORES = 8
RLOC = R // NCORES  # 256
NG = 4  # d-groups at partition offsets 0/32/64/96
RG = RLOC // NG  # 64 r's per group
RCH = 32  # r chunk size in routing phase
NCH = RLOC // RCH
ROUTING_ITERS = 3
F16 = mybir.dt.float16
F32 = mybir.dt.float32

LAST_EXEC_NS = None
_NC_CACHE = {}


def _emit_gen(tc, xt_ap, w_ap, xt16, w16, u):
    """Generate u_hat = x @ W into SBUF f16 tile u [128, O, C, RLOC]."""
    nc = tc.nc
    for g in range(NG):
        nc.gpsimd.dma_start(out=xt16[32 * g : 32 * g + D, :], in_=xt_ap)
        nc.gpsimd.dma_start(out=w16[32 * g : 32 * g + D], in_=w_ap[g * D : (g + 1) * D])

    with tc.tile_pool(name="psum", bufs=8, space="PSUM") as psump:
        i = 0
        for o in range(O):
            for g in range(NG):
                lhsT = xt16[32 * g : 32 * g + D, :]
                for h in range(RG // RCH):
                    ps = psump.tile([128, C, RCH], F32, tag="ps", name=f"ps_{o}_{g}_{h}")
                    rhs = w16[32 * g : 32 * g + D, o, :, h * RCH : (h + 1) * RCH]
                    nc.tensor.matmul(
                        ps, lhsT, rhs, start=True, stop=True, tile_position=(32 * g, 0)
                    )
                    r0 = g * RG + h * RCH
                    dst = u[:, o, :, r0 : r0 + RCH]
                    if i % 2 == 0:
                        nc.vector.tensor_copy(dst, ps)
                    else:
                        nc.scalar.copy(dst, ps)
                    i += 1


def _tree_reduce(nc, scr, lvl, dim, n, tag_prefix, dtype, rch):
    """Pairwise tree-sum along `dim` (size n -> 1). Returns [128, ..., rch] AP."""
    cnt = n
    while cnt > 1:
        half = cnt // 2
        if dim == 1:
            dst = scr.tile([128, half, C, rch], dtype, tag=f"{tag_prefix}{half}",
                           name=f"{tag_prefix}t{half}")
            pv = lvl.rearrange("p (o2 t) c r -> p o2 t c r", t=2)
            nc.vector.tensor_add(dst, pv[:, :, 0], pv[:, :, 1])
        elif dim == 2:
            dst = scr.tile([128, O, half, rch], dtype, tag=f"{tag_prefix}{half}",
                           name=f"{tag_prefix}t{half}")
            pv = lvl.rearrange("p o (c2 t) r -> p o c2 t r", t=2)
            nc.vector.tensor_add(dst, pv[:, :, :, 0], pv[:, :, :, 1])
        else:  # dim == 1 on a 3-dim [128, n, rch]
            dst = scr.tile([128, half, rch], dtype, tag=f"{tag_prefix}{half}",
                           name=f"{tag_prefix}t{half}")
            pv = lvl.rearrange("p (c2 t) r -> p c2 t r", t=2)
            nc.vector.tensor_add(dst, pv[:, :, 0], pv[:, :, 1])
        lvl = dst
        cnt = half
    return lvl


def _emit_routing(tc, u, out_ap):
    nc = tc.nc
    with (
        tc.tile_pool(name="state", bufs=1) as st,
        tc.tile_pool(name="scr", bufs=1) as scr,
        tc.tile_pool(name="ccdram", bufs=2, space="DRAM") as dramp,
    ):
        b_t = st.tile([128, O, RLOC], F16)
        e_t = st.tile([128, O, RLOC], F16)
        v_t = st.tile([128, C, RLOC], F16)
        zl = st.tile([128, O], F32)
        zg = st.tile([128, O], F32)

        for it in range(ROUTING_ITERS):
            if it > 0:
                # c_ij = exp(b) / sum_r exp(b); no max needed (|b| < 1).
                nc.scalar.activation(e_t, b_t, mybir.ActivationFunctionType.Exp)
                nc.vector.tensor_reduce(
                    zl, e_t, axis=mybir.AxisListType.X, op=mybir.AluOpType.add
                )
                cc_in = dramp.tile([128, O], F32, name=f"cc_in{it}")
                cc_out = dramp.tile([128, O], F32, name=f"cc_out{it}")
                nc.gpsimd.dma_start(out=cc_in, in_=zl)
                nc.gpsimd.collective_compute(
                    "AllReduce",
                    mybir.AluOpType.add,
                    replica_groups=[list(range(NCORES))],
                    ins=[cc_in.opt()],
                    outs=[cc_out.opt()],
                )
                nc.gpsimd.dma_start(out=zg, in_=cc_out)
                nc.vector.reciprocal(zg, zg)
                # e_t <- c_ij (f16), in place
                nc.vector.tensor_mul(
                    e_t, e_t, zg.unsqueeze(2).broadcast_to([128, O, RLOC])
                )

            for ch in range(NCH):
                r0 = ch * RCH
                rs = slice(r0, r0 + RCH)
                uc = u[:, :, :, rs]  # [128, O, C, RCH]

                if it == 0:
                    lvl = uc  # uniform c: plain o-sum, scale folded below
                else:
                    P = scr.tile([128, O, C, RCH], F16, tag="P", name="P")
                    cb = e_t[:, :, rs].unsqueeze(2).broadcast_to([128, O, C, RCH])
                    nc.vector.tensor_mul(P, uc, cb)
                    lvl = P
                lvl = _tree_reduce(nc, scr, lvl, 1, O, "T", F16, RCH)
                # lvl: [128, 1, C, RCH]
                if it == 0:
                    s_ch = scr.tile([128, C, RCH], F16, tag="sch", name="sch")
                    nc.vector.tensor_scalar_mul(s_ch, lvl[:, 0], 1.0 / R)
                else:
                    s_ch = lvl[:, 0]

                # squash factor = sqrt(ns)/(1+ns), ns = sum_c s^2 (fp32)
                sq = scr.tile([128, C, RCH], F32, tag="sq", name="sq")
                nc.vector.tensor_mul(sq, s_ch, s_ch)
                nsl = _tree_reduce(nc, scr, sq, 3, C, "q", F32, RCH)
                ns = nsl[:, 0]  # [128, RCH]
                rt = scr.tile([128, RCH], F32, tag="rt", name="rt")
                nc.scalar.sqrt(rt, ns)
                dn = scr.tile([128, RCH], F32, tag="dn", name="dn")
                nc.vector.tensor_scalar_add(dn, ns, 1.0)
                nc.vector.reciprocal(dn, dn)
                fc = scr.tile([128, RCH], F32, tag="fc", name="fc")
                nc.vector.tensor_mul(fc, rt, dn)

                vdst = v_t[:, :, rs]
                nc.vector.tensor_mul(
                    vdst, s_ch, fc.unsqueeze(1).broadcast_to([128, C, RCH])
                )

                if it < ROUTING_ITERS - 1:
                    # agreement a = sum_c u*v ; b += a
                    P2 = scr.tile([128, O, C, RCH], F16, tag="P", name="P2")
                    vb = vdst.unsqueeze(1).broadcast_to([128, O, C, RCH])
                    nc.vector.tensor_mul(P2, uc, vb)
                    al = _tree_reduce(nc, scr, P2, 2, C, "A", F16, RCH)
                    a_ch = al[:, :, 0]  # [128, O, RCH]
                    bdst = b_t[:, :, rs]
                    if it == 0:
                        nc.vector.tensor_copy(bdst, a_ch)
                    else:
                        nc.vector.tensor_add(bdst, bdst, a_ch)

        nc.gpsimd.dma_start(out=out_ap, in_=v_t)  # f16 -> f32 cast on DMA


def _build_nc():
    nc = bacc.Bacc(
        "TRN2",
        target_bir_lowering=False,
        debug=False,
        enable_asserts=False,
        num_devices=NCORES,
    )
    xt_d = nc.dram_tensor("xt", [D, B], F32, kind="ExternalInput")
    w_d = nc.dram_tensor("w", [NG * D, O, C, RG], F32, kind="ExternalInput")
    out_d = nc.dram_tensor("out", [B, C, RLOC], F32, kind="ExternalOutput")

    with tile.TileContext(nc) as tc:
        with (
            tc.tile_pool(name="const", bufs=1) as constp,
            tc.tile_pool(name="upool", bufs=1) as upool,
        ):
            xt16 = constp.tile([128, B], F16)
            u = upool.tile([128, O, C, RLOC], F16)
            with tc.tile_pool(name="wpool", bufs=1) as wpool:
                w16 = wpool.tile([128, O, C, RG], F16)
                _emit_gen(tc, xt_d.ap(), w_d.ap(), xt16, w16, u)
            _emit_routing(tc, u, out_d.ap())
    nc.compile()
    return nc


def _prep_inputs(x, route_weights):
    xt = np.ascontiguousarray(x.reshape(B, D).T.astype(np.float32))  # [D, B]
    w0 = np.asarray(route_weights).reshape(R, O, D, C)
    in_maps = []
    for i in range(NCORES):
        ws = w0[i * RLOC : (i + 1) * RLOC]  # (RLOC, O, D, C)
        ws = ws.reshape(NG, RG, O, D, C).transpose(0, 3, 2, 4, 1)  # (g, d, o, c, r)
        wprep = np.ascontiguousarray(ws.reshape(NG * D, O, C, RG).astype(np.float32))
        in_maps.append({"xt": xt, "w": wprep})
    return in_maps


def kernel(x, route_weights, trace=False):
    global LAST_EXEC_NS
    x = np.asarray(x, dtype=np.float32)
    route_weights = np.asarray(route_weights, dtype=np.float32)

    if "nc" not in _NC_CACHE:
        _NC_CACHE["nc"] = _build_nc()
    nc = _NC_CACHE["nc"]

    in_maps = _prep_inputs(x, route_weights)
    res = bass_utils.run_bass_kernel_spmd(
        nc, in_maps, core_ids=list(range(NCORES)), trace=trace
    )
    LAST_EXEC_NS = res.exec_time_ns

    shards = []
    for i in range(NCORES):
        o = res.results[i]["out"]  # [B, C, RLOC]
        shards.append(np.transpose(o, (0, 2, 1)))  # [B, RLOC, C]
    return np.concatenate(shards, axis=1).astype(np.float32)  # (B, R, C)
